# revision 14
# baseline (speedup 1.0000x reference)
"""CLIP encoder layer on 8 trn2 NeuronCores, pure data parallel over batch.

Layout strategy (per core, batch shard of 64 sequences = 4928 tokens):
  - x arrives token-major [T, 768] fp32.
  - LayerNorm runs token-major (tokens on partitions, bn_stats/bn_aggr),
    LN scale/bias folded into the downstream projection weights host-side.
    rstd computed as exp(-0.5*ln(var+eps)) on ACT so the LN path shares the
    natural_log_exp table set with attention's exp (no Sqrt table swaps,
    no DVE reciprocal).
  - Normalized activations are PE-transposed (bf16) to feature-major
    [768, N] for the projections (weights stationary, activations moving).
  - q/k PSUM drains (bias add + bf16 cast) run on ACT (Identity+bias),
    freeing DVE.
  - Attention per sequence (S=77), software-pipelined across the 4
    sequences of a superblock so the PE always has independent matmuls
    queued while softmax chains run on ACT/DVE:
      iter s: v_s, scores_s (6 matmuls/half into one PSUM bank) + one
      batched exp per half; tr_{s-1} (6 transposes into one PSUM bank,
      one batched copy); ctx_{s-2} (head pairs share a [128,77] PSUM tile
      via partition-offset writes, one cast per pair).
    Softmax: no max subtraction (scores bounded ~|2.5|); causal mask
    applied multiplicatively after exp.
  - O-projection runs with swapped operands (activations stationary) so
    its output comes out token-major, letting the residual add and LN2
    stay token-major with no full-tensor transposes.
  - FC2 runs feature-major (weights stationary, ff1 moving): 709k PE
    cycles instead of 884k; bias added during the ACT Identity drain
    (per-partition), then PE-transposed back to token-major for the
    residual add.
  - All matmuls in bf16 (fp32 PSUM accumulation); fp32 elsewhere.
    QuickGELU via ACT Silu: x*sigmoid(1.702x) = silu(1.702x)/1.702 with the
    1/1.702 folded into fc2 weights and the 1.702 into the ACT input scale.
"""

import os
import numpy as np
import ml_dtypes

D = 768
H = 12
HD = 64
S = 77
FF = 3072
EPS = 1e-5
N_CORES = 8
B_FULL = 512
BPC = B_FULL // N_CORES          # 64 sequences per core
T_CORE = BPC * S                 # 4928 tokens per core
G_SEQ = 4                        # sequences per superblock
SB = G_SEQ * S                   # 308 tokens per superblock


def build_program(T=T_CORE, G=G_SEQ, use_silu=True, stages="ABCDEF"):
    import concourse.bass as bass
    import concourse.bacc as bacc
    import concourse.mybir as mybir
    import concourse.tile as tile
    from concourse.masks import make_identity
    from contextlib import ExitStack

    f32 = mybir.dt.float32
    bf16 = mybir.dt.bfloat16
    f8 = mybir.dt.float8e4
    AX = mybir.AxisListType
    OP = mybir.AluOpType
    AF = mybir.ActivationFunctionType

    SBLK = G * S
    NSB = T // SBLK
    assert NSB * SBLK == T
    # token chunks within a superblock
    chunks = []
    off = 0
    while off < SBLK:
        w = min(128, SBLK - off)
        chunks.append((off, w))
        off += w

    nc = bacc.Bacc("TRN2", target_bir_lowering=False)

    x_d = nc.declare_dram_parameter("x", [T, D], f32, isOutput=False)
    wq_d = nc.declare_dram_parameter("wqT", [D, D], bf16, isOutput=False)
    wk_d = nc.declare_dram_parameter("wkT", [D, D], bf16, isOutput=False)
    wv_d = nc.declare_dram_parameter("wvT", [D, D], bf16, isOutput=False)
    wo_d = nc.declare_dram_parameter("woT", [D, D], bf16, isOutput=False)
    # fc1/fc2 weights arrive fp8 (e4m3, scaled x64 / x128) pre-packed for
    # DoubleRow: [p, blk, i, out] with contraction index k = blk*256+i*128+p
    wf1_d = nc.declare_dram_parameter("fc1T", [128, D // 256, 2, FF], f8,
                                      isOutput=False)
    wf2_d = nc.declare_dram_parameter("fc2T", [128, FF // 256, 2, D], f8,
                                      isOutput=False)
    qb_d = nc.declare_dram_parameter("qb", [D], f32, isOutput=False)
    kb_d = nc.declare_dram_parameter("kb", [D], f32, isOutput=False)
    vb_d = nc.declare_dram_parameter("vb", [D], f32, isOutput=False)
    ob_d = nc.declare_dram_parameter("ob", [D], f32, isOutput=False)
    f1b_d = nc.declare_dram_parameter("fc1b", [FF], f32, isOutput=False)
    f2b_d = nc.declare_dram_parameter("fc2b", [D], f32, isOutput=False)
    mask_d = nc.declare_dram_parameter("mask", [S, S], bf16, isOutput=False)
    out_d = nc.declare_dram_parameter("out", [T, D], f32, isOutput=True)

    with tile.TileContext(nc) as tc, ExitStack() as ctx:
        singles = ctx.enter_context(tc.tile_pool(name="singles", bufs=1))
        xpool = ctx.enter_context(tc.tile_pool(name="xpool", bufs=3))
        x2pool = ctx.enter_context(tc.tile_pool(name="x2pool", bufs=3))
        actpool = ctx.enter_context(tc.tile_pool(name="actpool", bufs=1))
        outpool = ctx.enter_context(tc.tile_pool(name="outpool", bufs=2))
        attnpool = ctx.enter_context(tc.tile_pool(name="attnpool", bufs=2))
        statpool = ctx.enter_context(tc.tile_pool(name="statpool", bufs=2))
        pspool = ctx.enter_context(tc.tile_pool(name="pspool", bufs=1, space="PSUM"))

        NCH = D // 128    # 6
        NFF = FF // 128   # 24

        # ---- tiny constants first (cheap DMAs, unblock LN immediately) ----
        qb_sb = singles.tile([128, D // 128], f32)
        kb_sb = singles.tile([128, D // 128], f32)
        f1b_sb = singles.tile([128, FF // 128], f32)
        f2b_sb = singles.tile([128, D // 128], f32)
        for sb_t, dr in ((qb_sb, qb_d), (kb_sb, kb_d), (f1b_sb, f1b_d),
                         (f2b_sb, f2b_d)):
            nc.sync.dma_start(out=sb_t, in_=dr[:].rearrange("(c p) -> p c", p=128))

        # free-axis biases broadcast to all 128 partitions (DMAs issued
        # after stage A(0)'s x loads; only needed from stage C on)
        ob_bc = singles.tile([128, D], f32)
        vb_bc = singles.tile([128, D], f32)
        mask_sb = singles.tile([S, S], bf16)

        def load_bcast():
            for sb_t, dr in ((ob_bc, ob_d), (vb_bc, vb_d)):
                srcap = bass.AP(tensor=dr[:].tensor, offset=dr[:].offset,
                                ap=[[0, 128]] + list(dr[:].ap))
                nc.sync.dma_start(out=sb_t, in_=srcap)
            nc.sync.dma_start(out=mask_sb, in_=mask_d[:])

        ident = singles.tile([128, 128], bf16)
        make_identity(nc, ident)

        eps_sb = singles.tile([128, 1], f32)
        nc.vector.memset(eps_sb, EPS)
        # eps/256 for LN2: sqrt((var+eps)/256) -> rstd x16 -> h2T scaled x16
        # so fc1's fp8 inputs use more of the e4m3 range (psum scale 16*64)
        eps256_sb = singles.tile([128, 1], f32)
        nc.vector.memset(eps256_sb, EPS / 256.0)

        # ---- weights (declared up front, DMAs issued after stage A(0)) ----
        wq_sb = singles.tile([128, D // 128, D], bf16)
        wk_sb = singles.tile([128, D // 128, D], bf16)
        wv_sb = singles.tile([128, D // 128, D], bf16)
        wo_sb = singles.tile([128, D // 128, D], bf16)
        wf1_sb = singles.tile([128, D // 256, 2, FF], f8)
        wf2_sb = singles.tile([128, FF // 256, 2, D], f8)

        def load_weights():
            for sb_t, dr in ((wq_sb, wq_d), (wk_sb, wk_d), (wv_sb, wv_d),
                             (wo_sb, wo_d)):
                nc.sync.dma_start(
                    out=sb_t, in_=dr[:].rearrange("(c p) o -> p c o", p=128))
            nc.sync.dma_start(out=wf1_sb, in_=wf1_d[:])
            nc.sync.dma_start(out=wf2_sb, in_=wf2_d[:])

        def ln_normalize(src_tile, w, tag, bufs=2, scale16=False):
            """token-major [w, 768] fp32 -> normalized bf16 htok tile."""
            stats = statpool.tile([128, 3, 6], f32, tag=f"stats{tag}", name=f"stats{tag}")
            mv = statpool.tile([128, 2], f32, tag=f"mv{tag}", name=f"mv{tag}")
            xg = src_tile[:w].rearrange("p (s f) -> p s f", f=256)
            for i in range(3):
                nc.vector.bn_stats(out=stats[:w, i, :], in_=xg[:, i, :])
            nc.vector.bn_aggr(out=mv[:w], in_=stats[:w])
            mean = mv[:w, 0:1]
            rstd = mv[:w, 1:2]
            if scale16:
                nc.scalar.activation(out=rstd, in_=rstd, func=AF.Sqrt,
                                     bias=eps256_sb[:w], scale=1.0 / 256.0)
            else:
                nc.scalar.activation(out=rstd, in_=rstd, func=AF.Sqrt,
                                     bias=eps_sb[:w], scale=1.0)
            nc.vector.reciprocal(out=rstd, in_=rstd)
            htok = statpool.tile([128, D], bf16, tag=f"htok{tag}", name=f"htok{tag}",
                                 bufs=bufs)
            nc.vector.tensor_scalar(out=htok[:w], in0=src_tile[:w],
                                    scalar1=mean, scalar2=rstd,
                                    op0=OP.subtract, op1=OP.mult)
            return htok

        def ln_transpose(htok, coff, w, hT, tag):
            for c in range(NCH):
                ps = bf_ps(f"trp{tag}")
                nc.tensor.transpose(ps[:, :w], htok[:w, c * 128:(c + 1) * 128],
                                    ident[:w, :w])
                nc.any.tensor_copy(out=hT[:, c, coff:coff + w], in_=ps[:, :w])

        def stage_A(isb):
            """load x, LN1 -> hT feature-major bf16."""
            t0 = isb * SBLK
            hT = actpool.tile([128, NCH, SBLK], bf16, tag="hT", name="hT", bufs=2)
            x_tiles = []
            for (coff, w) in chunks:
                x_tok = xpool.tile([128, D], f32, tag="xtok", name="xtok")
                nc.sync.dma_start(out=x_tok[:w], in_=x_d[t0 + coff: t0 + coff + w, :])
                x_tiles.append(x_tok)
                htok = ln_normalize(x_tok, w, "A")
                ln_transpose(htok, coff, w, hT, "A")
            return hT, x_tiles

        # All PSUM tiles are sized to exactly one 2KB bank so every tile is
        # bank-aligned (matmul outputs must not cross a bank boundary).
        def big_ps(name):
            return pspool.tile([128, 512], f32, tag="big", name=name, bufs=3)

        def bf_ps(name):
            # shared bf16 PSUM bank for transpose drains (attn, LN, fc2)
            return pspool.tile([128, 1024], bf16, tag="bfps", name=name, bufs=2)

        def stage_D_chunk(ci, ctxT, x_tiles, x2_tiles):
            coff, w = chunks[ci]
            x2 = x2pool.tile([128, D], f32, tag="x2tok", name="x2tok")
            for half in range(2):
                ps = big_ps("pso")
                for d in range(NCH):
                    nc.tensor.matmul(ps[:w, :384], lhsT=ctxT[:, d, coff:coff + w],
                                     rhs=wo_sb[:, d, half * 384:(half + 1) * 384],
                                     start=(d == 0), stop=(d == NCH - 1))
                sl = slice(half * 384, (half + 1) * 384)
                nc.vector.tensor_tensor(out=x2[:w, sl], in0=ps[:w, :384],
                                        in1=ob_bc[:w, sl], op=OP.add)
                nc.vector.tensor_tensor(out=x2[:w, sl], in0=x2[:w, sl],
                                        in1=x_tiles[ci][:w, sl], op=OP.add)
            x2_tiles.append(x2)

        cur = stage_A(0)
        load_bcast()
        load_weights()
        for isb in range(NSB):
            t0 = isb * SBLK
            hT, x_tiles = cur

            # ---- stage B: q/k projections (feature-major, bf16) ----
            # per-head layout [64, H, SBLK]: every scores lhsT starts at
            # partition 0 (a matmul with lhsT at partition base 64 AND a
            # free-offset PSUM output hangs the device)
            qT = actpool.tile([64, H, SBLK], bf16, tag="qT", name="qT")
            kT = actpool.tile([64, H, SBLK], bf16, tag="kT", name="kT")
            for dst, w_sb, b_sb in ((qT, wq_sb, qb_sb), (kT, wk_sb, kb_sb)):
                for c in range(NCH):
                    ps = big_ps("psqkv")
                    for d in range(NCH):
                        nc.tensor.matmul(ps[:, :SBLK],
                                         lhsT=w_sb[:, d, c * 128:(c + 1) * 128],
                                         rhs=hT[:, d, :], start=(d == 0),
                                         stop=(d == NCH - 1))
                    # bias add + bf16 cast + partition shift on ACT
                    nc.scalar.activation(out=dst[:, 2 * c, :], in_=ps[0:64, :SBLK],
                                         func=AF.Identity, bias=b_sb[0:64, c:c + 1],
                                         scale=1.0)
                    nc.scalar.activation(out=dst[:, 2 * c + 1, :],
                                         in_=ps[64:128, :SBLK],
                                         func=AF.Identity,
                                         bias=b_sb[64:128, c:c + 1], scale=1.0)

            # ---- stage C: attention, software-pipelined across sequences ----
            ctxT = actpool.tile([128, NCH, SBLK], bf16, tag="ctxT", name="ctxT")
            h2T = actpool.tile([128, NCH, SBLK], f8, tag="h2T", name="h2T")
            x2_tiles = []
            h2toks = []
            next_chunk = 0

            vtoks = [None] * G
            p_sbs = [None] * G
            attnTs = [None] * G

            def emit_v_scores(s):
                so = s * S
                vtok = attnpool.tile([S, H, HD], bf16, tag="vtok", name="vtok",
                                     bufs=3)
                for half in range(2):
                    psv = big_ps("psvtok")
                    for d in range(NCH):
                        nc.tensor.matmul(psv[:S, :384],
                                         lhsT=hT[:, d, so:so + S],
                                         rhs=wv_sb[:, d, half * 384:(half + 1) * 384],
                                         start=(d == 0), stop=(d == NCH - 1))
                    nc.vector.tensor_tensor(
                        out=vtok[:, half * 6:(half + 1) * 6, :],
                        in0=psv[:S, :384],
                        in1=vb_bc[:S, half * 384:(half + 1) * 384], op=OP.add)
                vtoks[s] = vtok
                p_sb = attnpool.tile([S, H, S], bf16, tag="p", name="p_sb", bufs=2)
                for half in range(2):
                    hh = half * 6
                    ps = pspool.tile([128, 512], f32, tag="scps", name="pssc",
                                     bufs=2)[:S, :468].rearrange(
                                         "p (i k) -> p i k", k=78)
                    for i in range(6):
                        h = hh + i
                        nc.tensor.matmul(ps[:, i, :S],
                                         lhsT=qT[:, h, so:so + S],
                                         rhs=kT[:, h, so:so + S],
                                         start=True, stop=True)
                    # one batched exp for 6 heads
                    nc.scalar.activation(out=p_sb[:, hh:hh + 6, :],
                                         in_=ps[:, :, :S], func=AF.Exp)
                p_sbs[s] = p_sb

            def emit_softmax_chain(s):
                p_sb = p_sbs[s]
                denom = statpool.tile([S, H], f32, tag="denom", name="denom")
                nc.vector.tensor_tensor(
                    out=p_sb[:], in0=p_sb[:],
                    in1=mask_sb[:, None, :].to_broadcast((S, H, S)), op=OP.mult)
                nc.vector.reduce_sum(out=denom[:], in_=p_sb[:], axis=AX.X)
                nc.vector.reciprocal(out=denom[:], in_=denom[:])
                nc.vector.tensor_tensor(
                    out=p_sb[:], in0=p_sb[:],
                    in1=denom[:, :, None].to_broadcast((S, H, S)), op=OP.mult)

            def emit_transposes(s):
                p_sb = p_sbs[s]
                attnT = attnpool.tile([S, H, S], bf16, tag="attnT", name="attnT")
                for half in range(2):
                    hh = half * 6
                    psa = bf_ps("psattnT")[:S, :468].rearrange(
                        "p (i k) -> p i k", k=78)
                    for i in range(6):
                        nc.tensor.transpose(psa[:, i, :S], p_sb[:, hh + i, :],
                                            ident[:S, :S])
                    nc.any.tensor_copy(out=attnT[:, hh:hh + 6, :],
                                       in_=psa[:, :, :S])
                attnTs[s] = attnT

            def emit_ctx(s):
                so = s * S
                vtok, attnT = vtoks[s], attnTs[s]
                # even heads -> ctxT partitions 0:64, odd heads -> 64:128;
                # matmul outputs stay at partition base 0, the copies shift.
                for j in range(2):
                    psc = pspool.tile([128, 512], f32, tag="scps",
                                      name="psctx", bufs=2)[:64, :468].rearrange(
                                          "p (c k) -> p c k", k=78)
                    for c in range(NCH):
                        h = 2 * c + j
                        nc.tensor.matmul(psc[:, c, :S],
                                         lhsT=vtok[:, h, :], rhs=attnT[:, h, :],
                                         start=True, stop=True)
                    nc.vector.tensor_copy(out=ctxT[j * 64:(j + 1) * 64, :,
                                                   so:so + S],
                                          in_=psc[:, :, :S])

            def emit_done_chunks(s_done):
                """emit O-proj + residual + LN2 for chunks fully covered by
                sequences 0..s_done."""
                nonlocal next_chunk
                done_tokens = (s_done + 1) * S
                while (next_chunk < len(chunks)
                       and chunks[next_chunk][0] + chunks[next_chunk][1]
                       <= done_tokens):
                    stage_D_chunk(next_chunk, ctxT, x_tiles, x2_tiles)
                    next_chunk += 1

            for s in range(G):
                emit_v_scores(s)
                if s >= 1:
                    emit_transposes(s - 1)
                if s >= 2:
                    emit_ctx(s - 2)
                    emit_done_chunks(s - 2)
                emit_softmax_chain(s)
            emit_transposes(G - 1)
            emit_ctx(G - 2)
            emit_done_chunks(G - 2)
            emit_ctx(G - 1)
            emit_done_chunks(G - 1)

            # E LayerNorms (deferred here so their Sqrt calls cluster with
            # stage A(+1)'s: one sqrt->exp->silu table rotation per superblock)
            for ci, (coff, w) in enumerate(chunks):
                h2toks.append(ln_normalize(x2_tiles[ci], w, "E", bufs=3,
                                           scale16=True))
            for ci, (coff, w) in enumerate(chunks):
                ln_transpose(h2toks[ci], coff, w, h2T, "E")

            # ---- prefetch next superblock's stage A before F: its sqrts join
            # the E cluster and its PE transposes run before F's matmuls, so
            # stage B of the next superblock starts with no LN stall ----
            if isb + 1 < NSB:
                cur = stage_A(isb + 1)

            # ---- stage F: MLP ----
            ff1 = actpool.tile([128, NFF, SBLK], f8, tag="ff1", name="ff1")
            DR = mybir.MatmulPerfMode.DoubleRow
            for f in range(NFF):
                ps = big_ps("psff")
                for blk in range(D // 256):
                    nc.tensor.matmul(ps[:, :SBLK],
                                     lhsT=wf1_sb[:, blk, :, f * 128:(f + 1) * 128],
                                     rhs=h2T[:, 2 * blk:2 * blk + 2, :],
                                     start=(blk == 0), stop=(blk == D // 256 - 1),
                                     perf_mode=DR)
                if use_silu:
                    # f1 = silu(1.702*ps + 1.702*b) = 1.702*quickgelu(ps+b);
                    # the 1/1.702 is folded into fc2T host-side.
                    nc.scalar.activation(out=ff1[:, f, :], in_=ps[:, :SBLK],
                                         func=AF.Silu,
                                         bias=f1b_sb[:, f:f + 1],
                                         scale=1.702 / 1024.0)
                else:
                    # CoreSim fallback: sigmoid + 2 DVE ops, same contract
                    sgt = statpool.tile([128, SBLK], bf16, tag="sgt", name="sgt",
                                        bufs=1)
                    nc.scalar.activation(out=sgt, in_=ps[:, :SBLK], func=AF.Sigmoid,
                                         bias=f1b_sb[:, f:f + 1],
                                         scale=1.702 / 1024.0)
                    nc.vector.tensor_scalar(out=ff1[:, f, :], in0=ps[:, :SBLK],
                                            scalar1=1.702 / 1024.0,
                                            scalar2=f1b_sb[:, f:f + 1],
                                            op0=OP.mult, op1=OP.add)
                    nc.vector.tensor_tensor(out=ff1[:, f, :], in0=ff1[:, f, :],
                                            in1=sgt, op=OP.mult)
            # fc2 feature-major: out_fm[c] = sum_f wf2[f,c].T @ ff1[f]
            of2 = actpool.tile([128, NCH, SBLK], bf16, tag="of2", name="of2")
            for c in range(NCH):
                ps = big_ps("psf2")
                for blk in range(FF // 256):
                    nc.tensor.matmul(ps[:, :SBLK],
                                     lhsT=wf2_sb[:, blk, :, c * 128:(c + 1) * 128],
                                     rhs=ff1[:, 2 * blk:2 * blk + 2, :],
                                     start=(blk == 0),
                                     stop=(blk == FF // 256 - 1), perf_mode=DR)
                nc.scalar.activation(out=of2[:, c, :], in_=ps[:, :SBLK],
                                     func=AF.Identity, bias=f2b_sb[:, c:c + 1],
                                     scale=1.0 / 128.0)
            # transpose back to token-major, add residual, store
            for ci, (coff, w) in enumerate(chunks):
                o_tok = outpool.tile([128, D], f32, tag="otok", name="otok")
                for half in range(2):
                    pstr = bf_ps("psf2tr")
                    for j in range(3):
                        c = half * 3 + j
                        nc.tensor.transpose(pstr[:w, j * 128:(j + 1) * 128],
                                            of2[:, c, coff:coff + w],
                                            ident[:, :])
                    sl = slice(half * 384, (half + 1) * 384)
                    nc.vector.tensor_tensor(out=o_tok[:w, sl],
                                            in0=pstr[:w, :384],
                                            in1=x2_tiles[ci][:w, sl], op=OP.add)
                nc.sync.dma_start(out=out_d[t0 + coff: t0 + coff + w, :],
                                  in_=o_tok[:w])

    nc.compile()
    return nc


def prep_shared(inputs):
    """Fold LN affine params / scale constants into weights -> shared in_map entries."""
    bf = ml_dtypes.bfloat16
    f32 = np.float32
    g = {k: np.asarray(v, dtype=np.float32) for k, v in inputs.items() if k != "x"}

    wqT = (g["ln1_w"][:, None] * g["qw"].T * 0.125).astype(bf)
    wkT = (g["ln1_w"][:, None] * g["kw"].T).astype(bf)
    wvT = (g["ln1_w"][:, None] * g["vw"].T).astype(bf)
    woT = np.ascontiguousarray(g["ow"].T).astype(bf)
    f8 = ml_dtypes.float8_e4m3fn
    # fc1/fc2 fp8 DoubleRow packing: k = blk*256 + i*128 + p -> [p, blk, i, :]
    fc1T = (g["ln2_w"][:, None] * g["fc1_w"].T * 64.0).astype(f8)
    fc1T = np.ascontiguousarray(
        fc1T.reshape(D // 256, 2, 128, FF).transpose(2, 0, 1, 3))
    fc2T = (g["fc2_w"].T / 1.702 * 128.0).astype(f8)
    fc2T = np.ascontiguousarray(
        fc2T.reshape(FF // 256, 2, 128, D).transpose(2, 0, 1, 3))

    qb = ((g["ln1_b"] @ g["qw"].T + g["qb"]) * 0.125).astype(f32)
    kb = (g["ln1_b"] @ g["kw"].T + g["kb"]).astype(f32)
    vb = (g["ln1_b"] @ g["vw"].T + g["vb"]).astype(f32)
    ob = g["ob"].astype(f32)
    fc1b = ((g["ln2_b"] @ g["fc1_w"].T + g["fc1_b"]) * 1.702).astype(f32)
    fc2b = g["fc2_b"].astype(f32)

    mask = np.tril(np.ones((S, S), np.float32)).astype(bf)   # [q, k], k<=q allowed

    return dict(wqT=wqT, wkT=wkT, wvT=wvT, woT=woT, fc1T=fc1T, fc2T=fc2T,
                qb=qb, kb=kb, vb=vb, ob=ob, fc1b=fc1b, fc2b=fc2b, mask=mask)


def prep_host_inputs(inputs):
    shared = prep_shared(inputs)
    x = np.asarray(inputs["x"], dtype=np.float32)
    in_maps = []
    for c in range(N_CORES):
        xc = np.ascontiguousarray(
            x[c * BPC:(c + 1) * BPC].reshape(T_CORE, D).astype(np.float32))
        in_maps.append(dict(shared, x=xc))
    return in_maps


_CACHED_NC = None


def _get_nc():
    global _CACHED_NC
    if _CACHED_NC is None:
        _CACHED_NC = build_program()
    return _CACHED_NC


def run(inputs, trace=False):
    from concourse.bass_utils import run_bass_kernel_spmd
    nc = _get_nc()
    in_maps = prep_host_inputs(inputs)
    res = run_bass_kernel_spmd(nc, in_maps, list(range(N_CORES)), trace=trace)
    outs = [np.asarray(res.results[c]["out"], dtype=np.float32).reshape(BPC, S, D)
            for c in range(N_CORES)]
    full = np.concatenate(outs, axis=0)
    return full, res


def kernel(**inputs):
    full, _ = run(inputs, trace=False)
    return full
